# revision 55
# baseline (speedup 1.0000x reference)
"""Trainium2 Bass kernel for the 3-metalayer forward-forward style MLP.

Distribution: the (10 labels x 512 batch) grid flattens to 5120 independent
rows; each of the 8 cores processes 640 rows (pure data parallelism, weights
replicated, no collectives).

Device-side algorithm (per core, rows R=640):
  - states kept feature-major [2048(part-chunks), R] as fp8e4m3 "snq" = 16*s
    (alpha=16 puts squares 256 s^2 in fp8's normal range); weights fp8e4m3
    prescaled x64 on host; all matmuls run in DoubleRow perf mode (2 k-chunks
    per instruction, 2x PE throughput), fp32 PSUM accumulate
  - the tile lowering emits one LDWEIGHTS per matmul; a DoubleRow LDWEIGHTS
    serializes with the matmul stream and costs ~60% of an FD=320 issue
    slot. term_pass therefore orders k-pairs OUTER / row-halves INNER so
    the two matmuls of a k-pair sit adjacent with identical weight slices,
    and a post-schedule pass (_dedup_ldweights) drops the duplicate loads;
    the surviving stream issues at the FD=320 streaming floor (~137ns/MM,
    ~97% of fp8 peak inside a pass)
  - normalized copies s = SX * snq/(16*||s||) in fp8 for most passes; the
    three pass boundaries where the consumer would stall on the producer's
    normalize chain instead matmul the producer's BARE relu output x (one
    ACT op after psum stop -- shortest possible boundary chain). The
    omitted "+c" const and the per-row 1/norm are both restored inside the
    consumer's eviction by a single scalar_tensor_tensor:
    (psum + V)*inv, where V = c @ W' is a host-precomputed per-output-
    feature column (V1/V2/V3). The +c add still runs off-critical-path to
    feed the square/norm reduction; squares quantize from fp8 (extra noise
    on sum-of-squares ~0.1%, inside the error budget)
  - row L2 norms: square on the otherwise-idle GpSimd engine (fp8,
    640-wide) + DoubleRow ones-matmul reduction over partitions (which also
    broadcasts the row sum to every partition); the reduce batch is NOT
    emitted inside the producing pass (its input chain outlives the
    eviction-deferral window and would stall the strict-FIFO PE queue) but
    early in the NEXT pass's stream, where the squares are long since ready
    and the 16 ones-matmuls share one deduped LDWEIGHTS; sqrt (ACT) +
    fast-reciprocal (DVE); eps dropped (biases bound ss >~ 0.1 per row);
    goodness = sum(s^2)/2048 falls out of the same reduction
  - t=0 terms with zero-state inputs are host-folded constants; the layer-1
    "pre" term (static overlay input) is computed once and reused all 3
    steps
"""

import numpy as np
import ml_dtypes

import concourse.bass as bass
import concourse.tile as tile
from concourse import bacc, mybir
from concourse.bass_utils import run_bass_kernel_spmd

BF = mybir.dt.bfloat16
F8 = mybir.dt.float8e4
F32 = mybir.dt.float32
NPBF = ml_dtypes.bfloat16
NPF8 = ml_dtypes.float8_e4m3
DR = mybir.MatmulPerfMode.DoubleRow

N_CORES = 8
P = 128
D_IN = 784
D_IN_PAD = 1024           # 8 * 128 (padded so KC1 is even: mixing
                          # DoubleRow and plain matmuls in one accumulation
                          # group costs far more in PE mode switches than
                          # the padded k-pair burns)
KC1 = 8                   # k-chunks for the 784->2048 matmul
KC = 16                   # k-chunks for 2048-contraction matmuls
MC = 16                   # output-feature chunks (2048 / 128)
H = 2048
B = 512
NL = 10
ROWS = NL * B             # 5120
R = ROWS // N_CORES       # 640 rows per core
RH = 320                  # psum row-chunk (2 per core-row-block)
EPS = 1e-4

SX = 64.0                 # fp8 scale on normalized states
SW = 64.0                 # fp8 scale on weights
AL = 16.0                 # alpha scale on raw states
SCL = AL / (SX * SW)      # eviction scale undoing fp8 scales, applying alpha

# bias/const column indices inside the packed [128, 15*16] bias tensor.
# V1/V2/V3 are the consumer-folded state constants: a raw-boundary pass
# matmuls the producer's bare relu output x (one ACT op after psum stop)
# and the omitted "+c" term is restored inside the eviction as the
# per-output-feature constant V = c @ W', added to the psum by the same
# scalar_tensor_tensor that applies the producer's 1/norm.
(B1PRE, B1POST, B1SELF, B2PRE, B2POST, B2SELF, B3PRE, B3SELF,
 C1, C2, C3, C3P, V1, V2, V3) = range(15)
NBIAS = 15

_NC_CACHE = {}


def _build_nc():
    """Build the single-core Tile program (same NEFF for all 8 cores)."""
    nc = bacc.Bacc("TRN2", target_bir_lowering=False, debug=False,
                   num_devices=N_CORES)

    hx_d = nc.dram_tensor("hxn", [P, KC1, R], F8, kind="ExternalInput")
    w_d = {
        "w1pre": nc.dram_tensor("w1pre", [MC, P, KC1, P], F8, kind="ExternalInput"),
    }
    for name in ("w1post", "w1self", "w2pre", "w2post", "w2self", "w3pre", "w3self"):
        w_d[name] = nc.dram_tensor(name, [MC, P, KC, P], F8, kind="ExternalInput")
    bias_d = nc.dram_tensor("biases", [P, NBIAS * MC], F32, kind="ExternalInput")
    g_d = nc.dram_tensor("g", [1, R], F32, kind="ExternalOutput")

    with tile.TileContext(nc) as tc:
        with (
            tc.tile_pool(name="consts", bufs=1) as consts,
            tc.tile_pool(name="states", bufs=1) as states,
            tc.tile_pool(name="wpool", bufs=12) as wpool,
            tc.tile_pool(name="epool", bufs=10) as epool,
            tc.tile_pool(name="sqpool", bufs=10) as sqpool,
            tc.tile_pool(name="small", bufs=3) as small,
            tc.tile_pool(name="mmps", bufs=6, space="PSUM") as mmps,
            tc.tile_pool(name="redps", bufs=2, space="PSUM") as redps,
        ):
            # startup order: first hx chunk + first weight block must land
            # before anything else so the PE starts within ~1.5us
            hx = states.tile([P, KC1, R], F8, tag="hxn")
            # hx rides the otherwise-idle Activation DMA queue so it lands in
            # parallel with the first weight block + biases on the sync queue
            nc.scalar.dma_start(out=hx[:, 0:2, :], in_=hx_d[:, 0:2, :])
            bias_sb = consts.tile([P, NBIAS * MC], F32)
            w0 = wpool.tile([P, KC1, P], F8, tag="w", name="w1pre0")
            nc.sync.dma_start(out=w0[:], in_=w_d["w1pre"][0])
            nc.scalar.dma_start(out=bias_sb[:], in_=bias_d[:])
            # progressive 2-chunk pieces: kpair j only needs chunks 2j,2j+1,
            # so the A-pass streams while later pieces are still in flight
            for kc in range(2, KC1, 2):
                nc.scalar.dma_start(out=hx[:, kc:kc + 2, :],
                                    in_=hx_d[:, kc:kc + 2, :])
            # [128, 2, 128] fp8 ones: M=128 DoubleRow ones-matmul reduces two
            # feature chunks over partitions at once AND broadcasts the row
            # sum-of-squares to every partition for free
            ones_red = consts.tile([P, P], BF)
            nc.vector.memset(ones_red[:], 1.0)
            ones8 = consts.tile([P, 2, P], F8)
            nc.vector.memset(ones8[:], 1.0)
            gacc = consts.tile([1, R], F32)

            # warm the PE HAM clock gate while the initial DMAs are in
            # flight so the real matmul stream starts at 2.4GHz
            warm_ps = mmps.tile([P, RH], F32, tag="mm", name="warm_ps")
            for _ in range(24):
                nc.tensor.matmul(warm_ps[:, :P], ones_red[:], ones_red[:],
                                 start=True, stop=True)
            At = states.tile([P, MC, R], F8, tag="A")
            s1 = states.tile([P, MC, R], F8, tag="s1")
            s2 = states.tile([P, MC, R], F8, tag="s2")
            s3 = states.tile([P, MC, R], F8, tag="s3")
            # bf16 scratch for the freshly combined state: squares and the
            # normalized copy quantize from bf16 (single fp8 quantization --
            # squaring an fp8 value doubles its relative error and corrupts
            # the row norms)
            snew = states.tile([P, MC, R], BF, tag="snew")
            # raw fp8 copies (16*s) of the three t0 states consumed by
            # raw+inv-folded boundary passes
            # raw fp8 states: bare relu outputs (x, matmulled by the raw
            # boundary consumers) and +c scratch copies (square/norm path)
            snq2 = states.tile([P, MC, R], F8, tag="snq2")
            snq3 = states.tile([P, MC, R], F8, tag="snq3")
            snq1s = states.tile([P, MC, R], F8, tag="snq1s")
            snq2s = states.tile([P, MC, R], F8, tag="snq2s")
            snq3s = states.tile([P, MC, R], F8, tag="snq3s")
            comb = states.tile([P, MC, R], BF, tag="comb")

            _red_uid = [0]

            def red_pair():
                # (psum_rh0, psum_rh1, pending sq pair-tiles). The ones-matmul
                # reduction over the sq tiles is NOT emitted inside the
                # producing pass: the sq chain (relu -> add -> square) has
                # more latency than the eviction deferral window, so an
                # inline reduce-matmul stalls the strict-FIFO PE queue.
                # Instead the batch is emitted early in the NEXT pass's
                # stream (emit fn from inv_defer), where the squares are
                # long since ready and the 16 ones-matmuls share one
                # deduped LDWEIGHTS.
                _red_uid[0] += 1
                u = _red_uid[0]
                return (redps.tile([P, RH], F32, tag="red", name=f"red{u}a"),
                        redps.tile([P, RH], F32, tag="red", name=f"red{u}b"),
                        [])

            def bias_ap(idx, mc):
                col = idx * MC + mc
                return bias_sb[:, col:col + 1]

            def rsl(rh):
                return slice(rh * RH, (rh + 1) * RH)

            def term_pass(wname, kcn, src, evict, w0_tile=None, defer=2,
                          pre_chunk=None, mid=None, mid2=None):
                """One linear term: stream weight blocks, accumulate psums,
                hand each [128, RH] psum chunk to `evict(mc, rh, ps)`.

                The k-pair loop is OUTER and the two row-halves INNER so the
                rh=0/rh=1 matmuls of one k-pair sit adjacent on the PE queue
                with identical weight slices: the post-schedule LDWEIGHTS
                dedup pass then drops every second weight load (DoubleRow
                LDWEIGHTS is serial with the matmul stream and costs ~60% of
                a matmul's issue slot).

                Evictions are emitted `defer` psum-groups late: the eviction
                chain (ACT relu -> DVE combine/square -> PE reduce-matmul)
                has ~1.5us of cross-engine latency, and emitting it inline
                makes the strict-FIFO PE queue stall on the reduce-matmul.
                Deferring places it behind independent matmul work."""
                wd = w_d[wname]
                pending = []
                for mc in range(MC):
                    if mc == 1 and mid is not None:
                        # previous pass's reduce batch: behind one mc-group
                        # of independent stream (covers the square drain)
                        mid()
                    if mc == 2 and mid2 is not None:
                        # its sqrt/recip (+ normalize muls): emitted a group
                        # later so the sqrt never blocks this pass's relu
                        # evictions on the strict-FIFO ACT queue while it
                        # waits for the reduce matmuls
                        mid2()
                    if pre_chunk is not None:
                        pre_chunk(mc)
                    if mc == 0 and w0_tile is not None:
                        wt = w0_tile
                    else:
                        wt = wpool.tile([P, kcn, P], F8, tag="w")
                        nc.sync.dma_start(out=wt[:], in_=wd[mc])
                    ps0 = mmps.tile([P, RH], F32, tag="mm")
                    ps1 = mmps.tile([P, RH], F32, tag="mm")
                    for kc in range(0, kcn, 2):
                        for rh, ps in ((0, ps0), (1, ps1)):
                            nc.tensor.matmul(
                                ps[:], wt[:, kc:kc + 2, :],
                                src[:, kc:kc + 2, rsl(rh)],
                                start=(kc == 0), stop=(kc == kcn - 2),
                                perf_mode=DR)
                    for rh, ps in ((0, ps0), (1, ps1)):
                        pending.append((mc, rh, ps))
                        if len(pending) > defer:
                            evict(*pending.pop(0))
                while pending:
                    evict(*pending.pop(0))

            def with_inv(evict_fn, inv_tile, vidx):
                """Raw-boundary eviction: pre-multiply the psum by the
                producer state's per-row 1/norm (broadcast over partitions),
                then run the standard eviction on the result. Lets the
                consumer pass matmul the raw snq without waiting for the
                producer's normalize chain."""
                def ev(mc, rh, ps):
                    e2 = epool.tile([P, RH], F32, tag="e", name="einv")
                    nc.vector.scalar_tensor_tensor(
                        e2[:], ps[:], bias_ap(vidx, mc),
                        inv_tile[:, rsl(rh)],
                        op0=mybir.AluOpType.add, op1=mybir.AluOpType.mult)
                    evict_fn(mc, rh, e2)
                return ev

            _sq_pair = [None]

            def sq_and_reduce(mc, rh, red, sq_eng=None, eager=False,
                              src=None, split_tail=True):
                """After both rh chunks of src[mc] (default snew) are
                written: square the full 640-wide chunk (fp8, 256 s^2) on
                the otherwise-idle GpSimd engine; every second chunk,
                accumulate two chunks' row sum-of-squares into the red
                psums via DoubleRow ones-matmuls."""
                s = snew if src is None else src
                if not eager and split_tail and mc >= MC - 2:
                    # tail chunks: square each row-half as soon as its add
                    # lands -- the next pass's reduce batch is gated on the
                    # LAST square, and a full-width op would start ~0.6us
                    # later than the rh0 half needs to
                    if mc % 2 == 0 and rh == 0:
                        _sq_pair[0] = sqpool.tile([P, 2, R], F8, tag="sq",
                                                  name="sqpair")
                    eng = sq_eng or (nc.gpsimd if mc % 2 else nc.vector)
                    eng.tensor_mul(_sq_pair[0][:, mc % 2, rsl(rh)],
                                   s[:, mc, rsl(rh)], s[:, mc, rsl(rh)])
                    if mc % 2 == 1 and rh == 1:
                        red[2].append(_sq_pair[0])
                    return
                if rh == 0 and not eager:
                    return
                if rh == 0 and eager:
                    if mc % 2 == 0:
                        _sq_pair[0] = sqpool.tile([P, 2, R], F8, tag="sq",
                                                  name="sqpair")
                    eng = nc.gpsimd if (eager and mc >= MC - 2) else sq_eng
                    (eng or sq_eng or nc.gpsimd).tensor_mul(
                        _sq_pair[0][:, mc % 2, rsl(0)],
                        s[:, mc, rsl(0)], s[:, mc, rsl(0)])
                    return
                if eager:
                    eng = nc.vector if (eager and mc >= MC - 2) else sq_eng
                    (eng or sq_eng or nc.gpsimd).tensor_mul(
                        _sq_pair[0][:, mc % 2, rsl(1)],
                        s[:, mc, rsl(1)], s[:, mc, rsl(1)])
                    if mc % 2 == 1:
                        for r in range(2):
                            nc.tensor.matmul(red[r][:], ones8[:],
                                             _sq_pair[0][:, 0:2, rsl(r)],
                                             start=(mc == 1),
                                             stop=(mc == MC - 1),
                                             perf_mode=DR)
                    return
                if mc % 2 == 0:
                    _sq_pair[0] = sqpool.tile([P, 2, R], F8, tag="sq",
                                              name="sqpair")
                sqt = _sq_pair[0]
                (sq_eng or (nc.gpsimd if mc % 2 else nc.vector)).tensor_mul(
                    sqt[:, mc % 2, :], s[:, mc, :], s[:, mc, :])
                if mc % 2 == 1:
                    red[2].append(sqt)

            def emit_red(red):
                """Emit the batched ones-matmul reduction over a pass's sq
                tiles (deferred to the following pass; see red_pair)."""
                sqts = red[2]
                for i, sqt in enumerate(sqts):
                    for r in range(2):
                        nc.tensor.matmul(red[r][:], ones8[:],
                                         sqt[:, 0:2, rsl(r)],
                                         start=(i == 0),
                                         stop=(i == len(sqts) - 1),
                                         perf_mode=DR)

            GSCL = 1.0 / (AL * AL * H)

            def gacc_from(red, goodness):
                # goodness scale folded into the accumulation so the final
                # output DMA reads gacc directly (no separate gout multiply
                # on the end-of-kernel critical path)
                for rh in range(2):
                    if goodness == "init":
                        nc.vector.tensor_scalar_mul(gacc[:, rsl(rh)],
                                                    red[rh][0:1, :], GSCL)
                    else:
                        nc.vector.scalar_tensor_tensor(
                            gacc[:, rsl(rh)], red[rh][0:1, :], GSCL,
                            gacc[:, rsl(rh)], op0=mybir.AluOpType.mult,
                            op1=mybir.AluOpType.add)

            def inv_defer(red, goodness, need_inv=True):
                """red[rh] will hold 256*ss per row, broadcast across all
                128 partitions by the M=128 ones-matmul. Allocates the inv
                tile now (so consumers can capture it) and returns
                (inv, emit): emit() places the reduce batch + sqrt + fast
                reciprocal; inv = SX/(16*sqrt(ss)) per row."""
                if need_inv:
                    nr = small.tile([P, R], F32, tag="nr", name="nr")
                    inv = small.tile([P, R], F32, tag="inv", name="inv")
                else:
                    nr = inv = None

                def emit_mms():
                    emit_red(red)

                def emit_inv():
                    if goodness:
                        gacc_from(red, goodness)
                    if not need_inv:
                        return
                    for rh in range(2):
                        # sqrt(red/SX^2) = AL*sqrt(ss)/SX; reciprocal per
                        # row-half so the first eviction (which reads only
                        # inv[:, rh0]) unblocks after one sqrt+recip instead
                        # of the full-width chain
                        nc.scalar.activation(
                            nr[:, rsl(rh)], red[rh][:],
                            mybir.ActivationFunctionType.Sqrt,
                            scale=1.0 / (SX * SX * SCL))
                        nc.vector.reciprocal_approx_fast(
                            out=inv[:, rsl(rh)], in_=nr[:, rsl(rh)])
                return inv, emit_mms, emit_inv

            def fin_muls_c(tgt, xq, cidx, scratch, inv):
                """Normalized fp8 copy of a raw-boundary state: tgt =
                (xq + c) * inv. Even chunks as one DVE STT from the bare
                relu output (one fewer quantization); odd chunks as a
                GpSimd mul on the scratch (xq + c) tile that the square
                path materialized anyway."""
                for mc in range(MC):
                    if mc < 2 or mc % 2 == 0:
                        nc.vector.scalar_tensor_tensor(
                            tgt[:, mc, :], xq[:, mc, :], bias_ap(cidx, mc),
                            inv[:], op0=mybir.AluOpType.add,
                            op1=mybir.AluOpType.mult)
                    else:
                        nc.gpsimd.tensor_mul(tgt[:, mc, :],
                                             scratch[:, mc, :], inv[:])

            def fin_muls(tgt, src, inv):
                """Normalized fp8 copy: tgt = src * inv, 640-wide and
                mc-ascending so a consumer pass's k-pair DR matmuls unblock
                two muls at a time. Alternates DVE/GpSimd so neither engine
                carries the whole 13us block while also serving evictions."""
                for mc in range(MC):
                    # first k-pair fully on the faster DVE so a directly
                    # dependent pass's first matmul group unblocks sooner
                    eng = nc.vector if mc < 2 or mc % 2 == 0 else nc.gpsimd
                    eng.tensor_mul(tgt[:, mc, :], src[:, mc, :], inv[:])

            def finale_mid(red, tgt, goodness):
                """Deferred finale: (inv, mid, mid2) closures emitting the
                reduce batch (mid, mc==1) and the inv chain + normalized
                fp8 copy of snew into `tgt` (mid2, mc==2) inside the next
                pass's stream."""
                inv, emit_mms, emit_inv = inv_defer(red, goodness)

                def mid2():
                    emit_inv()
                    fin_muls(tgt, snew, inv)
                return inv, emit_mms, mid2

            def evict_to(dst, bidx):
                def ev(mc, rh, ps):
                    if mc >= MC - 2 and rh == 1:
                        nc.vector.tensor_scalar(
                            dst[:, mc, rsl(rh)], ps[:], bias_ap(bidx, mc),
                            0.0, op0=mybir.AluOpType.add,
                            op1=mybir.AluOpType.max)
                    else:
                        nc.scalar.activation(
                            dst[:, mc, rsl(rh)], ps[:],
                            mybir.ActivationFunctionType.Relu,
                            bias=bias_ap(bidx, mc))
                return ev

            def evict_add_comb(bidx):
                def ev(mc, rh, ps):
                    e = epool.tile([P, RH], BF, tag="e")
                    nc.scalar.activation(
                        e[:], ps[:], mybir.ActivationFunctionType.Relu,
                        bias=bias_ap(bidx, mc))
                    nc.vector.tensor_add(comb[:, mc, rsl(rh)],
                                         e[:], comb[:, mc, rsl(rh)])
                return ev

            # ---- A = relu(hxn @ w1pre' + 0.7*b1pre), cached for all steps.
            # t0-n1 (snq1 = A + c1) is fused into the same pass so its
            # elementwise work overlaps the A matmuls chunk by chunk.
            # snq1 is written fp8 straight from the c1-add (2-link chain:
            # relu -> add) so the boundary consumer w2pre-t0 isn't gated on
            # a third serial copy; squares quantize from the fp8 raw state
            # (the extra fp8 noise on sum-of-squares is ~0.1%, far inside
            # the error budget).
            red = red_pair()

            def add_c(dst, src, cidx, mc, engs):
                """dst = src + c  (src, c >= 0), alternating between the two
                given engines by mc parity. "act" runs it on the scalar
                engine as Relu-with-bias (exact: both operands nonneg); each
                pass picks engines OFF its psum-recycling path (the psum's
                first reader -- ACT relu on plain passes, the DVE inv-mul on
                raw-boundary passes -- must never queue behind these adds)."""
                eng = engs[mc % 2]
                if eng == "act" and mc >= MC - 4:
                    eng = nc.vector
                if eng == "act":
                    nc.scalar.activation(dst, src,
                                         mybir.ActivationFunctionType.Relu,
                                         bias=bias_ap(cidx, mc))
                else:
                    eng.tensor_scalar_add(dst, src, bias_ap(cidx, mc))

            def ev_a(mc, rh, ps, red=red):
                # sqrt(SCL) is folded into hxn and w1pre host-side, so the
                # eviction is relu(ps + b) -- expressible as (add, max) on
                # DVE too; tail rh1 halves go there so the last chunks'
                # relus drain on two engines in parallel
                if mc >= MC - 2 and rh == 1:
                    nc.vector.tensor_scalar(
                        At[:, mc, rsl(rh)], ps[:], bias_ap(B1PRE, mc), 0.0,
                        op0=mybir.AluOpType.add, op1=mybir.AluOpType.max)
                else:
                    nc.scalar.activation(
                        At[:, mc, rsl(rh)], ps[:],
                        mybir.ActivationFunctionType.Relu,
                        bias=bias_ap(B1PRE, mc))
                add_c(snq1s[:, mc, rsl(rh)], At[:, mc, rsl(rh)], C1, mc,
                      ("act", nc.vector))
                sq_and_reduce(mc, rh, red, src=snq1s)

            term_pass("w1pre", KC1, hx, ev_a, w0_tile=w0)
            inv1, emms1, einv1 = inv_defer(red, None)

            # ---- t0, n2 / n3: single pre-term + const.
            # t1-n1's post/self term passes are wedged between them: they
            # only need s2(t0)/s1(t0) and don't touch comb (the t0 updates
            # don't use it), so their matmuls fill t0's serial-chain tails.
            def ev_t0(red, cidx, bpre, xq, scratch, engs):
                def ev(mc, rh, ps):
                    if mc >= MC - 2 and rh == 1:
                        nc.vector.tensor_scalar(
                            xq[:, mc, rsl(rh)], ps[:], bias_ap(bpre, mc),
                            0.0, op0=mybir.AluOpType.add,
                            op1=mybir.AluOpType.max)
                    else:
                        nc.scalar.activation(
                            xq[:, mc, rsl(rh)], ps[:],
                            mybir.ActivationFunctionType.Relu,
                            bias=bias_ap(bpre, mc))
                    add_c(scratch[:, mc, rsl(rh)], xq[:, mc, rsl(rh)],
                          cidx, mc, engs)
                    sq_and_reduce(mc, rh, red, src=scratch)
                return ev

            # w2pre-t0 consumes raw snq1: starts right as the A pass ends
            red = red_pair()
            term_pass("w2pre", KC, At,
                      with_inv(ev_t0(red, C2, B2PRE, snq2, snq2s,
                                     ("act", nc.vector)), inv1, V1),
                      mid=emms1, mid2=einv1, defer=4)
            inv2, emms2, einv2 = inv_defer(red, None)

            # w1post (t1 wedge) consumes raw snq2
            term_pass("w1post", KC, snq2,
                      with_inv(evict_to(comb, B1POST), inv2, V2),
                      mid=emms2, mid2=einv2, defer=4)
            # t0 normalized copies are produced lazily from the fp8 raw
            # copies, emitted where the DVE queue is light (the raw passes'
            # evictions carry the expensive psum-read inv muls)
            fin_muls_c(s1, At, C1, snq1s, inv1)
            term_pass("w1self", KC, s1, evict_add_comb(B1SELF))
            fin_muls_c(s2, snq2, C2, snq2s, inv2)

            red = red_pair()
            term_pass("w3pre", KC, s2,
                      ev_t0(red, C3, B3PRE, snq3, snq3s,
                            ("act", nc.vector)))
            inv3, emms3, einv3 = inv_defer(red, None)

            def n1_chunk(red):
                """Per-mc hook: n1 = A + comb (640-wide DVE add + sq/reduce),
                interleaved into the following pass's matmul stream so the
                PE never drains while DVE churns through the combine."""
                def hook(mc):
                    nc.vector.tensor_add(snew[:, mc, :], At[:, mc, :],
                                         comb[:, mc, :])
                    sq_and_reduce(mc, 1, red, split_tail=False)
                return hook

            # ---- t1 / t2
            carry_mms = carry_mid2 = None  # t1-n3's deferred finale
            for t in (1, 2):
                last = (t == 2)
                # n1 = A + relu(s2@w1post'+b) + relu(s1@w1self'+b)
                if t == 2:
                    term_pass("w1post", KC, s2, evict_to(comb, B1POST),
                              mid=carry_mms, mid2=carry_mid2)
                    term_pass("w1self", KC, s1, evict_add_comb(B1SELF))

                # n2 = relu(s1new@w2pre') + relu(s3@w2post') + relu(s2@w2self')
                # n1's combine rides inside the w2post pass; at t1 w2post
                # consumes raw snq3 (its producer's normalize chain just
                # finished emitting)
                red_n1 = red_pair()
                if t == 1:
                    term_pass("w2post", KC, snq3,
                              with_inv(evict_to(comb, B2POST), inv3, V3),
                              pre_chunk=n1_chunk(red_n1), mid=emms3,
                              mid2=einv3, defer=4)
                else:
                    term_pass("w2post", KC, s3, evict_to(comb, B2POST),
                              pre_chunk=n1_chunk(red_n1))
                invn1, emmsn1, mid2n1 = finale_mid(
                    red_n1, s1, "init" if last else None)
                term_pass("w2self", KC, s2, evict_add_comb(B2SELF),
                          mid=emmsn1, mid2=mid2n1)
                if t == 1:
                    # s3(t0)'s normalized copy, deferred past w2self so its
                    # muls don't crowd the DVE under w2self's evictions;
                    # first consumer (w3self) is still two passes away
                    fin_muls_c(s3, snq3, C3, snq3s, inv3)
                red = red_pair()

                def ev_n2(mc, rh, ps, red=red):
                    e = epool.tile([P, RH], BF, tag="e")
                    nc.scalar.activation(
                        e[:], ps[:], mybir.ActivationFunctionType.Relu,
                        bias=bias_ap(B2PRE, mc))
                    nc.vector.tensor_add(snew[:, mc, rsl(rh)],
                                         e[:], comb[:, mc, rsl(rh)])
                    sq_and_reduce(mc, rh, red)

                term_pass("w2pre", KC, s1, ev_n2)
                invn2, emmsn2, mid2n2 = finale_mid(
                    red, s2, "add" if last else None)

                # n3 = relu(s2new@w3pre') + c3p + relu(s3@w3self')
                term_pass("w3self", KC, s3, evict_to(comb, B3SELF),
                          mid=emmsn2, mid2=mid2n2)
                red = red_pair()

                def ev_n3(mc, rh, ps, red=red):
                    e = epool.tile([P, RH], BF, tag="e")
                    nc.scalar.activation(
                        e[:], ps[:], mybir.ActivationFunctionType.Relu,
                        bias=bias_ap(B3PRE, mc))
                    nc.vector.scalar_tensor_tensor(
                        snew[:, mc, rsl(rh)], e[:], bias_ap(C3P, mc),
                        comb[:, mc, rsl(rh)],
                        op0=mybir.AluOpType.add, op1=mybir.AluOpType.add)
                    sq_and_reduce(
                        mc, rh, red,
                        sq_eng=(nc.vector if mc % 2 else nc.gpsimd)
                        if last else None,
                        eager=last)

                term_pass("w3pre", KC, s2, ev_n3)
                if last:
                    # eager reduction already accumulated inline; goodness
                    # only
                    gacc_from(red, "add")
                else:
                    invn3, carry_mms, carry_mid2 = finale_mid(
                        red, s3, None)

            # ---- goodness out (already scaled in gacc_from)
            nc.sync.dma_start(out=g_d[:], in_=gacc[:])

    _dedup_ldweights(nc)
    nc.compile()
    return nc


def _dedup_ldweights(nc):
    """Drop InstLdweights whose weight AP + mode match the immediately
    preceding load on the PE queue (the tile lowering emits one per matmul,
    even for back-to-back matmuls sharing a weight block). The removed
    load's waits move to the following matmult; `move_matmul_waits_to_
    ldweights` in compile() then re-homes the surplus onto the kept load."""
    removed = 0
    for fn in nc.m.functions:
        for bb in fn.blocks:
            insts = bb.instructions
            out = []
            last_sig = None
            pending = None
            for inst in insts:
                if isinstance(inst, mybir.InstLdweights):
                    sig = (str(inst.ins[0]), str(inst.perf_mode),
                           str(inst.is_transpose), str(inst.tile_position),
                           str(inst.tile_size))
                    if sig == last_sig:
                        pending = inst
                        removed += 1
                        continue
                    last_sig = sig
                elif isinstance(inst, mybir.InstMatmult):
                    if pending is not None:
                        inst.add_sync_dependencies_from(
                            pending.sync_dependency_set_copy())
                        inst.add_nosync_dependencies_from(
                            pending.nosync_dependency_set_copy())
                        pending = None
                out.append(inst)
            assert pending is None, "removed LDWEIGHTS with no next matmult"
            if len(out) != len(insts):
                bb.instructions = out
    return removed


def _block_weight(w, scale, kcn):
    """[2048, d_in] float32 -> [MC, P, kcn, P] fp8e4m3 blocked for linear DMA:
    host_w[mc, p, kc, m] = scale * SW * W[mc*128+m, kc*128+p]."""
    w = np.asarray(w, dtype=np.float32) * (scale * SW)
    din = w.shape[1]
    if din < kcn * P:
        w = np.pad(w, ((0, 0), (0, kcn * P - din)))
    blk = w.reshape(MC, P, kcn, P).transpose(0, 3, 2, 1)
    return np.ascontiguousarray(blk.astype(NPF8))


def _col(v):
    """[2048] -> [128, 16] (partition-major bias layout)."""
    return np.asarray(v, dtype=np.float32).reshape(MC, P).T


def prepare_inputs(inputs):
    """Host prep: overlay+normalize Hx, prescale/block weights, pack biases.
    Returns (shared_map, per_core_hx list)."""
    x = np.asarray(inputs["x"], dtype=np.float32)
    mx = x.max()
    base = x.copy()
    base[:, :NL] = 0.0
    hx = np.tile(base[None, :, :], (NL, 1, 1))
    for l in range(NL):
        hx[l, :, l] = mx
    hx = hx.reshape(ROWS, D_IN)
    n = np.linalg.norm(hx, axis=1, keepdims=True)
    hxn = hx / (n + EPS) * (SX * 0.0625)
    hxn = np.pad(hxn, ((0, 0), (0, D_IN_PAD - D_IN)))

    per_core_hx = []
    for c in range(N_CORES):
        h = hxn[c * R:(c + 1) * R].T            # [1024, 640]
        h = h.reshape(KC1, P, R).transpose(1, 0, 2)
        per_core_hx.append(np.ascontiguousarray(h.astype(NPF8)))

    shared = {
        "w1pre": _block_weight(inputs["w1_pre"], 0.7 * 0.0625, KC1),
        "w1post": _block_weight(inputs["w1_post"], 0.7 * 0.0625, KC),
        "w1self": _block_weight(inputs["w1_self"], 0.3 * 0.0625, KC),
        "w2pre": _block_weight(inputs["w2_pre"], 0.7 * 0.0625, KC),
        "w2post": _block_weight(inputs["w2_post"], 0.7 * 0.0625, KC),
        "w2self": _block_weight(inputs["w2_self"], 0.3 * 0.0625, KC),
        "w3pre": _block_weight(inputs["w3_pre"], 0.7 * 0.0625, KC),
        "w3self": _block_weight(inputs["w3_self"], 0.3 * 0.0625, KC),
    }

    relu = lambda a: np.maximum(np.asarray(a, dtype=np.float32), 0.0)

    cols = np.empty((P, NBIAS * MC), dtype=np.float32)
    # all bias/const columns carry the alpha=16 state scaling (relu positive
    # homogeneity: AL*relu(u+b) = relu(AL*u + AL*b))
    vals = {
        B1PRE: AL * 0.7 * np.asarray(inputs["b1_pre"], np.float32),
        B1POST: AL * 0.7 * np.asarray(inputs["b1_post"], np.float32),
        B1SELF: AL * 0.3 * np.asarray(inputs["b1_self"], np.float32),
        B2PRE: AL * 0.7 * np.asarray(inputs["b2_pre"], np.float32),
        B2POST: AL * 0.7 * np.asarray(inputs["b2_post"], np.float32),
        B2SELF: AL * 0.3 * np.asarray(inputs["b2_self"], np.float32),
        B3PRE: AL * 0.7 * np.asarray(inputs["b3_pre"], np.float32),
        B3SELF: AL * 0.3 * np.asarray(inputs["b3_self"], np.float32),
        C1: AL * (0.7 * relu(inputs["b1_post"]) + 0.3 * relu(inputs["b1_self"])),
        C2: AL * (0.7 * relu(inputs["b2_post"]) + 0.3 * relu(inputs["b2_self"])),
        C3: AL * (0.7 * relu(inputs["b3_post"]) + 0.3 * relu(inputs["b3_self"])),
        C3P: AL * 0.7 * relu(inputs["b3_post"]),
    }
    # consumer-folded constants: V = (SW-scaled consumer weight) @ (AL*c) --
    # restores the "+c" term a raw-boundary pass omits from its moving
    # operand, added to the psum inside the eviction STT
    w = lambda n: np.asarray(inputs[n], np.float32)
    vals[V1] = 0.7 * 0.0625 * SW * (w("w2_pre") @ vals[C1])
    vals[V2] = 0.7 * 0.0625 * SW * (w("w1_post") @ vals[C2])
    vals[V3] = 0.7 * 0.0625 * SW * (w("w2_post") @ vals[C3])
    for idx, v in vals.items():
        cols[:, idx * MC:(idx + 1) * MC] = _col(v)
    shared["biases"] = np.ascontiguousarray(cols)

    return shared, per_core_hx


def run(inputs, trace=False):
    shared, per_core_hx = prepare_inputs(inputs)
    if "nc" not in _NC_CACHE:
        _NC_CACHE["nc"] = _build_nc()
    nc = _NC_CACHE["nc"]
    in_maps = [dict(shared, hxn=per_core_hx[c]) for c in range(N_CORES)]
    res = run_bass_kernel_spmd(nc, in_maps, core_ids=list(range(N_CORES)),
                               trace=trace)
    g = np.concatenate([res.results[c]["g"][0] for c in range(N_CORES)])
    out = g.reshape(NL, B).T.astype(np.float32)
    return np.ascontiguousarray(out), res


def kernel(**inputs):
    out, _ = run(inputs, trace=False)
    return out

